# revision 5
# baseline (speedup 1.0000x reference)
"""Trainium2 Bass kernel for nn_Conv2d_14147622273082.

Conv2d 3x3, stride 1, pad 1: x [8, 320, 64, 64] f32, hf8-coded weights
w_bits [320, 320, 3, 3] i32 (codes 0..255), bias codes b_bits [320] i32.
out = conv2d(x, hf8_decode(w_bits)) + hf8_decode(b_bits).

Strategy: data-parallel over batch (1 image per NeuronCore, 8 cores).
hf8 decode runs on the host (weights are replicated and every hf8 value
is exactly representable in fp16; the f16 weight DMA, 1.84MB, is smaller
than the raw i32 codes) — on-device DVE decode was latency/throughput
bound and gated the matmul stream.

x-stationary dataflow: out[pix, co] accumulates in PSUM tiles of
[128 pixels, 320 couts]; the stationary operand is a 128-pixel window of
the image, the moving operand the weights [Cin_chunk, 320] (N=320
exact, M=128 always full). The previous w-stationary layout wasted half
the PE array on the 64-wide Cout tail: 282624 vs 235520 column-cycles.
HW-measured stream rate: 136 ns/MM at N=320 (LDWEIGHTS of a fresh
stationary is fully hidden).

The stationary must be a single-free-dim AP, so the padded image is
stored as three width-64 copies S_kw (kw = 0,1,2), horizontally
pre-shifted by kw-1 with zero fill, 66 rows (top/bottom zero): the
window for kernel position (kh, kw) over output rows (h0, h0+1) is the
contiguous 128-element slice S_kw[(h0+kh)*64 : +128]. S1 (center) is
exactly the raw image, so it is DMA'd straight from HBM; S0/S2 are
Scalar-engine shifted copies of S1. Kernel positions run in the order
[1,4,7, 0,3,6, 2,5,8] so the stream starts on DMA-direct S1 data.

Cin=320 splits into chunks (128, 128, 64). The 64-wide tail packs kernel
positions in pairs on partitions (0:64 | 64:128) — the upper half holds
the partner position's shifted image so one K=128 matmul covers two
positions: packs PKA=[S0|S1] (pairs 0-1, 6-7), PKB=[S1|S2] (4-5),
PKC=[S2|S0 shifted up a row] (2-3, plus the pos-8 solo on its lower
half). The S1-planes of the packs are DMA-direct; the rest are DVE
copies. 9 positions -> 4 pairs + 1 solo: 23 accumulations per PSUM tile.
Host pre-pairs the tail weights into [128, 5, 320] f16.

x is host-cast to fp16 (device math was fp16 anyway); output returns
[pix, co] fp16 and is transposed / upcast on host.
"""

import numpy as np

import concourse.bass as bass
import concourse.tile as tile
from concourse import bacc, mybir
from concourse.bass_utils import run_bass_kernel_spmd

B, CIN, COUT, H, W = 8, 320, 320, 64, 64
PIX = H * W  # 4096
P = 128
HS = H + 2  # 66 rows in the shifted copies (top/bottom zero)
ROWS_PER_TILE = 2  # 2 rows of 64 = 128 pixels per PSUM tile
N_TILES = H // ROWS_PER_TILE  # 32
BATCH = 8  # PSUM banks in flight
N_WARM = 36
QR = H // 4  # DMA quarter: 16 image rows

POS_ORDER = [1, 4, 7, 0, 3, 6, 2, 5, 8]  # S1-planes first, then S0, S2
TAIL_PAIRS = [(0, 1), (2, 3), (4, 5), (6, 7)]  # (lower, upper), pos = kh*3+kw
N_ACC = 2 * 9 + len(TAIL_PAIRS) + 1  # 23 accumulations per PSUM tile

F16 = mybir.dt.float16
F32 = mybir.dt.float32


def build():
    from concourse.tile_rust import add_dep_helper

    nc = bacc.Bacc(
        "TRN2", target_bir_lowering=False, debug=False, enable_partition_id=False
    )
    x_d = nc.dram_tensor("x", [CIN, H, W], F16, kind="ExternalInput")
    w_d = nc.dram_tensor("w9", [2 * P, 9, COUT], F16, kind="ExternalInput")
    wp_d = nc.dram_tensor("wp", [P, 5, COUT], F16, kind="ExternalInput")
    b_d = nc.dram_tensor("b", [P, COUT], F32, kind="ExternalInput")
    out_d = nc.dram_tensor("out", [PIX, COUT], F16, kind="ExternalOutput")

    with tile.TileContext(nc) as tc:
        with (
            tc.tile_pool(name="persist", bufs=1) as persist,
            tc.tile_pool(name="stage", bufs=1) as stage,
            tc.tile_pool(name="outsb", bufs=4) as outsb,
            tc.tile_pool(name="psum", bufs=1, space="PSUM") as psum_pool,
        ):
            # ---- SBUF tiles ----
            wt = [
                persist.tile([P, 9, COUT], F16, tag=f"wl{c}", name=f"wl{c}")
                for c in range(2)
            ]
            wpair = persist.tile([P, 5, COUT], F16, tag="wpair", name="wpair")
            # shifted padded copies: S[c][kw][p, r, col] = x[p, r-1, col+kw-1]
            S = [
                [
                    persist.tile([P, HS, W], F16, tag=f"s{c}{kw}", name=f"s{c}{kw}")
                    for kw in range(3)
                ]
                for c in range(2)
            ]
            # tail packs (lower 0:64 | upper 64:128 partitions)
            pka = persist.tile([P, HS, W], F16, tag="pka", name="pka")
            pkb = persist.tile([P, HS, W], F16, tag="pkb", name="pkb")
            pkc = persist.tile([P, HS, W], F16, tag="pkc", name="pkc")
            biasb = persist.tile([P, COUT], F32, tag="biasb", name="biasb")

            # ---- input DMAs, deadline order, one in-order queue ----
            # w chunk slices land pos-group first (S1 group = slots 0:3)
            cs, ce = 256, 320
            nc.sync.dma_start(wt[0][:, 0:3], w_d[0:P, 0:3])
            for q in range(2):  # S1[0] image rows 0:32, straight from HBM
                nc.sync.dma_start(
                    S[0][1][:, 1 + q * QR : 1 + (q + 1) * QR, :],
                    x_d[0:P, q * QR : (q + 1) * QR],
                )
            nc.sync.dma_start(wt[0][:, 3:6], w_d[0:P, 3:6])
            for q in range(2):
                nc.sync.dma_start(
                    S[1][1][:, 1 + q * QR : 1 + (q + 1) * QR, :],
                    x_d[P : 2 * P, q * QR : (q + 1) * QR],
                )
            nc.sync.dma_start(wt[0][:, 6:9], w_d[0:P, 6:9])
            nc.sync.dma_start(wt[1][:], w_d[P : 2 * P])
            # tail S1 planes, straight from HBM
            nc.sync.dma_start(pkb[0:64, 1 : H + 1, :], x_d[cs:ce])
            nc.sync.dma_start(pka[64:128, 1 : H + 1, :], x_d[cs:ce])
            nc.sync.dma_start(wpair[:], wp_d[:, :])
            for q in range(2, 4):
                nc.sync.dma_start(
                    S[0][1][:, 1 + q * QR : 1 + (q + 1) * QR, :],
                    x_d[0:P, q * QR : (q + 1) * QR],
                )
                nc.sync.dma_start(
                    S[1][1][:, 1 + q * QR : 1 + (q + 1) * QR, :],
                    x_d[P : 2 * P, q * QR : (q + 1) * QR],
                )
            nc.sync.dma_start(biasb[:], b_d[:, :])

            # ---- PE warmup: keep TensorE busy (HAM at 8/8) through the
            # prologue so the real stream starts at 2.4 GHz ----
            wsrc = stage.tile([P, P], F16, tag="wsrc", name="wsrc")
            nc.vector.memset(wsrc[:], 0.0)
            warm_ps = psum_pool.tile([P, 512], F32, tag="acc0", name="warm_ps")
            for _ in range(N_WARM):
                nc.tensor.matmul(
                    warm_ps[:, 0:P], wsrc[:], wsrc[:], start=True, stop=True
                )

            # ---- border zeros (GpSimd: keeps Scalar/DVE free) ----
            g = nc.gpsimd
            for c in range(2):
                for kw in range(3):
                    g.memset(S[c][kw][:, 0:1, :], 0.0)
                    g.memset(S[c][kw][:, HS - 1 : HS, :], 0.0)
                g.memset(S[c][0][:, 1 : HS - 1, 0:1], 0.0)
                g.memset(S[c][2][:, 1 : HS - 1, W - 1 : W], 0.0)
            for pk in (pka, pkb):  # rows 0,65 zero on both halves
                g.memset(pk[:, 0:1, :], 0.0)
                g.memset(pk[:, HS - 1 : HS, :], 0.0)
            g.memset(pka[0:64, 1 : HS - 1, 0:1], 0.0)  # lower = S0t
            g.memset(pkb[64:128, 1 : HS - 1, W - 1 : W], 0.0)  # upper = S2t
            # pkc lower = S2t: rows 0,65 + col 63; upper = S0t shifted up a
            # row: rows 64,65 + col 0
            g.memset(pkc[0:64, 0:1, :], 0.0)
            g.memset(pkc[0:64, HS - 1 : HS, :], 0.0)
            g.memset(pkc[0:64, 1 : HS - 1, W - 1 : W], 0.0)
            g.memset(pkc[64:128, H : HS, :], 0.0)
            g.memset(pkc[64:128, 0:H, 0:1], 0.0)

            # ---- S0/S2 shifted placement on Scalar (warm the Copy table
            # first), chained in deadline order ----
            warm = stage.tile([P, 1], F16, tag="warm", name="warm")
            nc.vector.memset(warm[:], 0.0)
            chain = [nc.scalar.copy(warm[:], warm[:])]
            for hf in range(2):
                r0, r1 = 1 + 32 * hf, 33 + 32 * hf
                for c in range(2):
                    src = S[c][1][:, r0:r1]
                    chain.append(
                        nc.scalar.copy(
                            S[c][0][:, r0:r1, 1:W], src[:, :, 0 : W - 1]
                        )
                    )
                    chain.append(
                        nc.scalar.copy(
                            S[c][2][:, r0:r1, 0 : W - 1], src[:, :, 1:W]
                        )
                    )
            for a, b2 in zip(chain[1:], chain[:-1]):
                add_dep_helper(a.ins, b2.ins, sync=False, reason="cast order")

            # ---- remaining tail-pack planes on DVE ----
            vchain = []
            lo, hi = pkb[0:64, 1 : H + 1, :], pka[64:128, 1 : H + 1, :]
            vchain.append(
                nc.vector.tensor_copy(pka[0:64, 1 : H + 1, 1:W], lo[:, :, 0 : W - 1])
            )
            vchain.append(
                nc.vector.tensor_copy(pkc[0:64, 1 : H + 1, 0 : W - 1], lo[:, :, 1:W])
            )
            vchain.append(
                nc.vector.tensor_copy(
                    pkb[64:128, 1 : H + 1, 0 : W - 1], hi[:, :, 1:W]
                )
            )
            vchain.append(
                nc.vector.tensor_copy(pkc[64:128, 0:H, 1:W], hi[:, :, 0 : W - 1])
            )
            for a, b2 in zip(vchain[1:], vchain[:-1]):
                add_dep_helper(a.ins, b2.ins, sync=False, reason="pack order")

            # ---- matmuls: out[pix, co] += x_win[ci, pix].T @ w[ci, co] ----
            Sf = [[s.rearrange("p h w -> p (h w)") for s in Sc] for Sc in S]
            pkaf = pka.rearrange("p h w -> p (h w)")
            pkbf = pkb.rearrange("p h w -> p (h w)")
            pkcf = pkc.rearrange("p h w -> p (h w)")
            # pair -> (pack, row offset kh of the pair's lower position)
            pair_src = {(0, 1): (pkaf, 0), (2, 3): (pkcf, 0),
                        (4, 5): (pkbf, 1), (6, 7): (pkaf, 2)}

            for batch in range(N_TILES // BATCH):
                tiles = list(range(BATCH * batch, BATCH * (batch + 1)))
                accs = {
                    t: psum_pool.tile(
                        [P, 512], F32, tag=f"acc{t % BATCH}", name=f"acc_{t}"
                    )
                    for t in tiles
                }
                acc_k = {t: 0 for t in tiles}

                def mm(t, lhsT, rhs, accs=accs, acc_k=acc_k):
                    nc.tensor.matmul(
                        accs[t][:, :COUT], lhsT, rhs,
                        start=(acc_k[t] == 0), stop=(acc_k[t] == N_ACC - 1),
                    )
                    acc_k[t] += 1

                # phase 1: full Cin chunks, position-major in POS_ORDER so
                # the stream needs each S plane / w slice only as it lands
                for ci in range(2):
                    for slot, pos in enumerate(POS_ORDER):
                        kh, kw = divmod(pos, 3)
                        for t in tiles:
                            o = (2 * t + kh) * W
                            mm(t, Sf[ci][kw][:, o : o + P], wt[ci][:, slot, :])
                # phase 2: tail pairs + solo, tile-major; close each tile's
                # group and run its epilogue so it overlaps the stream
                for t in tiles:
                    for j, pr in enumerate(TAIL_PAIRS):
                        src, kh = pair_src[pr]
                        o = (2 * t + kh) * W
                        mm(t, src[:, o : o + P], wpair[:, j, :])
                    o = (2 * t + 2) * W
                    mm(t, pkcf[0:64, o : o + P], wpair[0:64, 4, :])
                    assert acc_k[t] == N_ACC
                    osb = outsb.tile([P, COUT], F16, tag="osb", name=f"osb{t}")
                    nc.vector.tensor_tensor(
                        osb[:], accs[t][:, :COUT], biasb[:], mybir.AluOpType.add
                    )
                    nc.sync.dma_start(out_d[t * P : (t + 1) * P, :], osb[:])

    nc.compile()
    return nc


_NC_CACHE = None


def _get_nc():
    global _NC_CACHE
    if _NC_CACHE is None:
        _NC_CACHE = build()
    return _NC_CACHE


def _hf8_decode_np(bits):
    """Decode 8-bit hf8 codes (1 sign, 4 exp, 3 man; bias 14) to float64."""
    bits = np.asarray(bits).astype(np.int64)
    sign = np.where((bits >> 7) & 1 == 1, -1.0, 1.0)
    exp = (bits >> 3) & 0xF
    man = (bits & 0x7).astype(np.float64)
    normal = np.exp2(exp - 14.0) * (1.0 + man / 8.0)
    subnormal = np.exp2(-13.0) * (man / 8.0)
    return sign * np.where(exp == 0, subnormal, normal)


def _prep_in_maps(x, w_bits, b_bits):
    # host-side decode + relayout (weights are replicated across cores)
    wd = _hf8_decode_np(w_bits)  # [co, ci, kh, kw] f64
    w9 = wd.transpose(1, 2, 3, 0).reshape(CIN, 9, COUT)  # [ci, pos, co]
    w9p = np.ascontiguousarray(
        w9[: 2 * P][:, POS_ORDER, :], dtype=np.float16
    )  # [256, slot, co]
    wp = np.zeros((P, 5, COUT), np.float16)
    for j, (pa, pb) in enumerate(TAIL_PAIRS):
        wp[0:64, j] = w9[2 * P :, pa]
        wp[64:128, j] = w9[2 * P :, pb]
    wp[0:64, 4] = w9[2 * P :, 8]
    b2 = np.ascontiguousarray(
        np.broadcast_to(
            _hf8_decode_np(b_bits).reshape(1, COUT), (P, COUT)
        ),
        dtype=np.float32,
    )
    xf = x.astype(np.float16)
    return [
        {"x": xf[i], "w9": w9p, "wp": wp, "b": b2}
        for i in range(B)
    ]


def kernel(x, w_bits, b_bits):
    nc = _get_nc()
    in_maps = _prep_in_maps(x, w_bits, b_bits)
    res = run_bass_kernel_spmd(nc, in_maps, core_ids=list(range(B)), trace=False)
    out = np.stack([res.results[i]["out"] for i in range(B)])  # [B, PIX, COUT]
    return out.transpose(0, 2, 1).reshape(B, COUT, H, W).astype(np.float32)


if __name__ == "__main__":
    rng = np.random.default_rng(0)
    x = rng.standard_normal((B, CIN, H, W)).astype(np.float32)
    w_bits = rng.integers(0, 256, (COUT, CIN, 3, 3)).astype(np.int32)
    b_bits = rng.integers(0, 256, (COUT,)).astype(np.int32)
    out = kernel(x, w_bits, b_bits)
    print("out", out.shape, out.dtype, float(np.abs(out).mean()))
